# revision 19
# baseline (speedup 1.0000x reference)
"""MultiHeadAttention (QK-RMSNorm + RoPE + causal) Trainium2 Bass kernel, v3.

Sharding: 8 cores = 2 batches x 4 head-groups (4 heads each).
Each core computes a partial (2048, 1024) output (its heads' contribution
through the output projection); host sums the 4 group-partials per batch.

v3 changes vs v2:
- Per-i score->exp->AV pipeline with double-buffered PSUM score tiles
  (the 2-i groups at bufs=1 serialized PE behind ACT, ~132us PE idle).
- Causal diag mask as pre-exp DVE add on PSUM (gpsimd dispatch latency sat
  between exp and AV on the critical path).
- softmax denominators via reciprocal_approx_fast (DVE iterative divide on
  a (1,512) row cost 3.3us/call).
- q/k transposes via DMA xbar (dma_start_transpose) instead of PE matmul
  transposes: PE runs pure back-to-back matmuls and stays HAM-warm.
"""

import math
import sys
from contextlib import ExitStack

import numpy as np

sys.path.insert(0, "/opt/trn_rl_repo")

import concourse.bass as bass  # noqa: E402
import concourse.bacc as bacc  # noqa: E402
import concourse.tile as tile  # noqa: E402
from concourse import mybir  # noqa: E402

B = 2
T = 2048
D = 1024
H = 16
HD = 64
G = 4  # heads per core
NCORES = 8
NT = T // 128  # 16 t-tiles
ND = D // 128  # 8 d-chunks
EPS = 1e-6
ROPE_BASE = 10000.0
MASK_NEG = -30000.0

F32 = mybir.dt.float32
BF16 = mybir.dt.bfloat16
AX = mybir.AxisListType
ALU = mybir.AluOpType
ACTF = mybir.ActivationFunctionType

NPBF16 = mybir.dt.np(BF16)

DEBUG = False

_ACT_PATCHED = False


def _patch_act_tables():
    """Route every activation fn this kernel uses (Exp/Ln/Square/Copy) to the
    single `natural_log_exp_and_others` set so only one ACT_TABLE_LOAD is
    ever emitted (the greedy per-fn chooser otherwise thrashes exp<->ln)."""
    global _ACT_PATCHED
    if _ACT_PATCHED:
        return
    from concourse.hw_specs import get_activation_tables as _orig

    target = "natural_log_exp_and_others"
    strip = {
        ACTF.Exp, ACTF.Ln, ACTF.Square, ACTF.Copy, ACTF.Identity,
    }

    def patched(arch):
        tabs = _orig(arch)
        if target in tabs:
            keep = tabs[target]
            for name, fns in tabs.items():
                if name != target:
                    for f in strip:
                        if f in keep:
                            fns.discard(f)
        return tabs

    bacc.get_activation_tables = patched
    _ACT_PATCHED = True


def build_program():
    _patch_act_tables()
    nc = bacc.Bacc(None, target_bir_lowering=False, debug=False)

    with tile.TileContext(nc) as tc:
        ctx = ExitStack()
        with ctx:
            dram = ctx.enter_context(tc.tile_pool(name="dram", bufs=1, space="DRAM"))
            xT_d = dram.tile([ND, 128, T], BF16, kind="ExternalInput", name="xT", uniquify=False)
            wqkv_d = dram.tile([ND, 128, 772], BF16, kind="ExternalInput", name="wqkv", uniquify=False)
            wo_d = dram.tile([2, 128, D], BF16, kind="ExternalInput", name="wo", uniquify=False)
            rc_d = dram.tile([NT, 128, 128], BF16, kind="ExternalInput", name="rc", uniquify=False)
            rs_d = dram.tile([NT, 128, 128], BF16, kind="ExternalInput", name="rs", uniquify=False)
            mtri_d = dram.tile([128, 128], F32, kind="ExternalInput", name="mtri", uniquify=False)
            vones_d = dram.tile([128, NT, G], BF16, kind="ExternalInput", name="vones", uniquify=False)
            outp_d = dram.tile([NT, 128, D], BF16, kind="ExternalOutput", name="outp", uniquify=False)
            if DEBUG:
                qkt_dbg = dram.tile([128, 4, T], BF16, kind="ExternalOutput", name="qkt_dbg", uniquify=False)
                va_dbg = dram.tile([128, NT, 260], BF16, kind="ExternalOutput", name="va_dbg", uniquify=False)
                ot01_dbg = dram.tile([128, T], BF16, kind="ExternalOutput", name="ot01_dbg", uniquify=False)
                ot23_dbg = dram.tile([128, T], BF16, kind="ExternalOutput", name="ot23_dbg", uniquify=False)
                rv_dbg = dram.tile([128, NT, 8], BF16, kind="ExternalOutput", name="rv_dbg", uniquify=False)

            # ---- persistent SBUF (whole kernel) ----
            persist = ctx.enter_context(tc.tile_pool(name="persist", bufs=1))
            # transposed q/k: quarters = qt01, qt23, kt01, kt23
            qkt = persist.tile([128, 4, T], BF16)
            v_all = persist.tile([128, NT, 260], BF16)     # 4x(64 V cols + ones)
            ot01 = persist.tile([128, T], BF16)            # heads 0,1 attn out (hd x T)
            ot23 = persist.tile([128, T], BF16)
            wo_s = persist.tile([128, 2, D], BF16)
            mtri_s = persist.tile([128, 128], F32)
            rc_s = persist.tile([128, NT, 128], BF16)      # cos tables (q|k)
            rs_s = persist.tile([128, NT, 128], BF16)      # sin tables (q|k)
            stats = persist.tile([128, NT, 8], F32)
            rv = persist.tile([128, NT, 8], BF16)          # rsqrt(mean+eps)
            eps_s = persist.tile([128, 1], F32)
            xt_s = [persist.tile([128, T], BF16, name=f"xt{c}") for c in range(ND)]
            w_s = persist.tile([128, ND, 772], BF16)

            nc.vector.memset(eps_s, EPS)
            nc.sync.dma_start(out=mtri_s, in_=mtri_d)
            nc.sync.dma_start(out=rc_s, in_=rc_d.rearrange("t p n -> p t n"))
            nc.sync.dma_start(out=rs_s, in_=rs_d.rearrange("t p n -> p t n"))
            nc.sync.dma_start(out=w_s, in_=wqkv_d.rearrange("c p n -> p c n"))
            nc.sync.dma_start(out=wo_s, in_=wo_d.rearrange("h p n -> p h n"))
            for c in range(ND):
                nc.sync.dma_start(out=xt_s[c], in_=xT_d[c])
            vones_cols = bass.AP(
                tensor=v_all.tensor,
                offset=v_all.offset + 64,
                ap=[v_all.ap[0], [260, NT], [65, G]])
            nc.sync.dma_start(out=vones_cols, in_=vones_d)

            # ================= Phase 1: proj + rmsnorm + rope + transpose
            p1 = ExitStack()
            with p1:
                work1 = p1.enter_context(tc.tile_pool(name="work1", bufs=2))
                ps_qk = p1.enter_context(tc.tile_pool(name="ps_qk", bufs=2, space="PSUM"))
                ps_v = p1.enter_context(tc.tile_pool(name="ps_v", bufs=2, space="PSUM"))

                for it in range(NT):
                    qkp = ps_qk.tile([128, 512], F32, tag="qk")
                    vp = ps_v.tile([128, 260], F32, tag="v")
                    for c in range(ND):
                        lhs = xt_s[c][:, it * 128:(it + 1) * 128]
                        nc.tensor.matmul(qkp, lhs, w_s[:, c, 0:512],
                                         start=(c == 0), stop=(c == ND - 1))
                        nc.tensor.matmul(vp, lhs, w_s[:, c, 512:772],
                                         start=(c == 0), stop=(c == ND - 1))
                    # drain Q|K to bf16 (ACT), squares to f32 (ACT)
                    qraw = work1.tile([128, 512], BF16, tag="qraw")
                    nc.scalar.copy(qraw, qkp)
                    scr = work1.tile([128, 512], F32, tag="scr")
                    nc.scalar.activation(out=scr, in_=qkp, func=ACTF.Square)
                    # V drain: 4x64 value cols (ones cols DMA'd once)
                    vdst = bass.AP(
                        tensor=v_all.tensor,
                        offset=v_all[:, it, :].offset,
                        ap=[v_all.ap[0], [65, 4], [1, 64]])
                    vsrc = bass.AP(
                        tensor=vp.tensor,
                        offset=vp.offset,
                        ap=[vp.ap[0], [65, 4], [1, 64]])
                    nc.scalar.copy(vdst, vsrc)
                    # per-seg sumsq -> rsqrt(mean+eps) = exp(-0.5*ln(.))
                    nc.vector.tensor_reduce(
                        out=stats[:, it, :],
                        in_=scr.rearrange("p (s e) -> p s e", e=64),
                        axis=AX.X, op=ALU.add)
                    nc.scalar.activation(out=stats[:, it, :], in_=stats[:, it, :],
                                         func=ACTF.Ln, scale=1.0 / HD, bias=eps_s)
                    nc.scalar.activation(out=rv[:, it, :], in_=stats[:, it, :],
                                         func=ACTF.Exp, scale=-0.5)
                    # rope: t1 = qraw*cos + rot(qraw)*sin   (tables carry norm_w
                    # + 0.125 q scaling)
                    t1 = work1.tile([128, 512], BF16, tag="t1")
                    rot = work1.tile([128, 512], BF16, tag="rot")
                    pq = qraw.ap[0]
                    # t1 = qraw * ctab  (one op: q cols use rc[:,it,0:64], k cols 64:128)
                    nc.vector.tensor_tensor(
                        out=bass.AP(tensor=t1.tensor, offset=t1.offset,
                                    ap=[t1.ap[0], [256, 2], [64, 4], [1, 64]]),
                        in0=bass.AP(tensor=qraw.tensor, offset=qraw.offset,
                                    ap=[pq, [256, 2], [64, 4], [1, 64]]),
                        in1=bass.AP(tensor=rc_s.tensor, offset=rc_s[:, it, :].offset,
                                    ap=[rc_s.ap[0], [64, 2], [0, 4], [1, 64]]),
                        op=ALU.mult)
                    # rot = rot_half_swap(qraw) * stab, q half then k half
                    for w in range(2):
                        c0 = 256 * w
                        nc.vector.tensor_tensor(
                            out=bass.AP(tensor=rot.tensor, offset=rot.offset + c0,
                                        ap=[rot.ap[0], [64, 4], [32, 2], [1, 32]]),
                            in0=bass.AP(tensor=qraw.tensor,
                                        offset=qraw.offset + c0 + 32,
                                        ap=[pq, [64, 4], [-32, 2], [1, 32]]),
                            in1=bass.AP(tensor=rs_s.tensor,
                                        offset=rs_s[:, it, :].offset + 64 * w,
                                        ap=[rs_s.ap[0], [0, 4], [32, 2], [1, 32]]),
                            op=ALU.mult)
                    nc.vector.tensor_add(out=t1, in0=t1, in1=rot)
                    # rms scale per 64-seg (gpsimd; DVE is hot in phase 1)
                    nc.gpsimd.tensor_tensor(
                        out=bass.AP(tensor=t1.tensor, offset=t1.offset,
                                    ap=[t1.ap[0], [64, 8], [1, 64]]),
                        in0=bass.AP(tensor=t1.tensor, offset=t1.offset,
                                    ap=[t1.ap[0], [64, 8], [1, 64]]),
                        in1=bass.AP(tensor=rv.tensor, offset=rv[:, it, :].offset,
                                    ap=[rv.ap[0], [1, 8], [0, 64]]),
                        op=ALU.mult)
                    # transpose 4 col-blocks -> qkt quarters via DMA xbar
                    for cb in range(4):
                        nc.sync.dma_start_transpose(
                            out=qkt[:, cb, it * 128:(it + 1) * 128],
                            in_=t1[:, cb * 128:(cb + 1) * 128])

            # ================= Phases 2+3: attention (j-outer) + out proj
            p23 = ExitStack()
            with p23:
                ptpool = p23.enter_context(tc.tile_pool(name="ptpool", bufs=3))
                nrm = p23.enter_context(tc.tile_pool(name="nrm", bufs=2))
                outpool = p23.enter_context(tc.tile_pool(name="outpool", bufs=2))
                ps_sg = p23.enter_context(tc.tile_pool(name="ps_sg", bufs=2, space="PSUM"))
                ps_op = p23.enter_context(tc.tile_pool(name="ps_op", bufs=1, space="PSUM"))
                ps_o2 = p23.enter_context(tc.tile_pool(name="ps_o2", bufs=2, space="PSUM"))

                for j in range(4):
                    kmax = 4 * (j + 1)
                    jsl = slice(j * 512, (j + 1) * 512)
                    for pr in range(2):  # head pair (2pr, 2pr+1)
                        qt = qkt[:, pr, :]
                        kt = qkt[:, 2 + pr, :]
                        opA = ps_op.tile([65, 512], F32, tag="opA")
                        opB = ps_op.tile([65, 512], F32, tag="opB")
                        hA, hB = 2 * pr, 2 * pr + 1
                        for i in range(kmax):
                            r = i - 4 * j
                            sgA = ps_sg.tile([128, 512], F32, tag="sgA")
                            sgB = ps_sg.tile([128, 512], F32, tag="sgB")
                            ptA = ptpool.tile([128, 512], BF16, tag="ptA")
                            ptB = ptpool.tile([128, 512], BF16, tag="ptB")
                            # head pair scores (row tiles at base partitions 0/64)
                            nc.tensor.matmul(
                                sgA, kt[0:64, i * 128:(i + 1) * 128],
                                qt[0:64, jsl], start=True, stop=True)
                            nc.tensor.matmul(
                                sgB, kt[64:128, i * 128:(i + 1) * 128],
                                qt[64:128, jsl], start=True, stop=True)
                            if r >= 0:  # diag block: additive causal mask
                                dsl = slice(128 * r, 128 * (r + 1))
                                nc.vector.tensor_add(
                                    out=sgA[:, dsl], in0=sgA[:, dsl], in1=mtri_s)
                                nc.vector.tensor_add(
                                    out=sgB[:, dsl], in0=sgB[:, dsl], in1=mtri_s)
                            nc.scalar.activation(out=ptA, in_=sgA, func=ACTF.Exp)
                            nc.scalar.activation(out=ptB, in_=sgB, func=ACTF.Exp)
                            c0 = max(0, 128 * r)
                            nc.tensor.matmul(
                                opA[:, c0:512],
                                v_all[:, i, hA * 65:hA * 65 + 65],
                                ptA[:, c0:512],
                                start=(i == 0), stop=(i == kmax - 1))
                            nc.tensor.matmul(
                                opB[:, c0:512],
                                v_all[:, i, hB * 65:hB * 65 + 65],
                                ptB[:, c0:512],
                                start=(i == 0), stop=(i == kmax - 1))
                        # normalize: ot[pb:pb+64, jsl] = op[0:64] * recip(den)
                        otp = (ot01, ot23)[pr]
                        for (op_, pb) in ((opA, 0), (opB, 64)):
                            den = nrm.tile([1, 512], F32, tag="den")
                            nc.vector.tensor_copy(den, op_[64:65, :])
                            rec = nrm.tile([1, 512], F32, tag="rec")
                            nc.vector.reciprocal_approx_fast(out=rec, in_=den)
                            bc = nrm.tile([64, 512], F32, tag="bc")
                            nc.gpsimd.partition_broadcast(bc, rec)
                            nc.vector.tensor_mul(
                                out=otp[pb:pb + 64, jsl],
                                in0=op_[0:64, :], in1=bc)
                    # ---- phase 3 for this j's 4 t-tiles
                    for it in range(4 * j, 4 * j + 4):
                        osb = outpool.tile([128, D], BF16, tag="osb")
                        for n in range(2):
                            pso = ps_o2.tile([128, 512], F32, tag="pso")
                            nc.tensor.matmul(
                                pso, ot01[:, it * 128:(it + 1) * 128],
                                wo_s[:, 0, n * 512:(n + 1) * 512],
                                start=True, stop=False)
                            nc.tensor.matmul(
                                pso, ot23[:, it * 128:(it + 1) * 128],
                                wo_s[:, 1, n * 512:(n + 1) * 512],
                                start=False, stop=True)
                            nc.vector.tensor_copy(osb[:, n * 512:(n + 1) * 512], pso)
                        nc.sync.dma_start(out=outp_d[it], in_=osb)
                if DEBUG:
                    nc.sync.dma_start(out=qkt_dbg, in_=qkt)
                    nc.sync.dma_start(out=va_dbg, in_=v_all)
                    nc.sync.dma_start(out=ot01_dbg, in_=ot01)
                    nc.sync.dma_start(out=ot23_dbg, in_=ot23)
                    nc.sync.dma_start(out=rv_dbg, in_=rv)

    nc.compile()
    return nc


_PROGRAM = None


def _get_program():
    global _PROGRAM
    if _PROGRAM is None:
        _PROGRAM = build_program()
    return _PROGRAM


def make_inputs_for_core(core, x, Wq, Wk, Wv, Wo, q_norm_w, k_norm_w):
    b, g = core // 4, core % 4
    xT = np.ascontiguousarray(np.asarray(x[b]).T).reshape(ND, 128, T)
    wq = np.asarray(Wq[:, 256 * g:256 * (g + 1)], np.float64)
    wk = np.asarray(Wk[:, 256 * g:256 * (g + 1)], np.float64)
    wv = np.asarray(Wv[:, 256 * g:256 * (g + 1)], np.float64)
    wvp = np.zeros((D, 260), np.float64)
    for h in range(G):
        wvp[:, h * 65:h * 65 + 64] = wv[:, h * 64:(h + 1) * 64]
    wqkv = np.concatenate([wq, wk, wvp], axis=1).reshape(ND, 128, 772)
    # stacked head pairs for the output projection (rows g*256 .. g*256+256)
    wo = np.asarray(Wo[256 * g:256 * (g + 1), :], np.float64).reshape(2, 128, D)

    inv_freq = 1.0 / (ROPE_BASE ** (np.arange(0, HD, 2, dtype=np.float64) / HD))
    tarr = np.arange(T, dtype=np.float64)
    fr = np.outer(tarr, inv_freq)
    cos, sin = np.cos(fr), np.sin(fr)

    def tables(w, scale):
        c = np.empty((T, HD), np.float64)
        s = np.empty((T, HD), np.float64)
        c[:, :32] = cos * w[:32] * scale
        c[:, 32:] = cos * w[32:] * scale
        s[:, :32] = -sin * w[32:] * scale
        s[:, 32:] = sin * w[:32] * scale
        return c, s

    qw = np.asarray(q_norm_w, np.float64)
    kw = np.asarray(k_norm_w, np.float64)
    qc, qs = tables(qw, 0.125)
    kc, ks = tables(kw, 1.0)
    rc = np.concatenate([qc, kc], axis=1).reshape(NT, 128, 128)
    rs = np.concatenate([qs, ks], axis=1).reshape(NT, 128, 128)

    kp = np.arange(128)[:, None]
    qf = np.arange(128)[None, :]
    mtri = np.where(qf >= kp, 0.0, MASK_NEG)
    vones = np.ones((128, NT, G))

    bf = NPBF16
    return {
        "xT": xT.astype(bf), "wqkv": wqkv.astype(bf), "wo": wo.astype(bf),
        "rc": rc.astype(bf), "rs": rs.astype(bf),
        "mtri": mtri.astype(np.float32),
        "vones": vones.astype(bf),
    }


def run_on_hw(inputs, trace=False):
    from concourse.bass_utils import run_bass_kernel_spmd
    nc = _get_program()
    in_maps = [make_inputs_for_core(c, **inputs) for c in range(NCORES)]
    res = run_bass_kernel_spmd(nc, in_maps, list(range(NCORES)), trace=trace)
    parts = [res.results[c]["outp"].astype(np.float32).reshape(T, D)
             for c in range(NCORES)]
    out = np.stack([sum(parts[0:4]), sum(parts[4:8])]).astype(np.float32)
    return out, res


def kernel(**inputs):
    out, _ = run_on_hw(inputs, trace=False)
    return out
